# revision 14
# baseline (speedup 1.0000x reference)
import sys

import numpy as np

if "/opt/trn_rl_repo" not in sys.path:
    sys.path.insert(0, "/opt/trn_rl_repo")

import concourse.bacc as bacc
import concourse.bass_isa as bass_isa
import concourse.mybir as mybir
import concourse.tile as tile
from concourse.bass_utils import run_bass_kernel_spmd

# Problem constants (hardcoded per harness contract)
B, C, K = 32768, 1000, 5
N_CORES = 8
ROWS = B // N_CORES          # 4096 rows per core
P = 128                      # partitions
NT = ROWS // P               # 32 row-tiles per core; row r = p*NT + t
FP32 = mybir.dt.float32
# Tapered wave sizes (tiles per DMA chunk): big chunks stream at full rate;
# a long single-tile tail lets the scalar engine (≈1.2 µs/tile incl. the
# pipelined accumulator read) consume tiles as they land (≈1.27 µs/tile), so
# only ONE tile's exp remains after the stream ends.
WS = [4, 4, 4, 4, 4, 4, 2, 1, 1, 1, 1, 1, 1]
assert sum(WS) == NT
NEG = -10000.0               # exp(NEG) == 0: masks invalid/duplicate labels


def _build_kernel():
    nc = bacc.Bacc()
    x = nc.declare_dram_parameter("x", [ROWS, C], FP32, isOutput=False)
    gv = nc.declare_dram_parameter("gv", [P, NT * K], FP32, isOutput=False)
    out = nc.declare_dram_parameter("out", [1, 1], FP32, isOutput=True)

    with tile.TileContext(nc) as tc:
        with tc.tile_pool(name="pp", bufs=1) as pp:
            x_all = pp.tile([P, NT * C], FP32)   # full per-core slice, exp'd in place
            gv_sb = pp.tile([P, NT * K], FP32)   # complementary-label logits (host-gathered)
            ge = pp.tile([P, NT * K], FP32)      # exp of the above
            denom = pp.tile([P, NT], FP32)
            numer = pp.tile([P, NT], FP32)
            rec = pp.tile([P, NT], FP32)
            loss = pp.tile([P, NT], FP32)
            lsum = pp.tile([P, 1], FP32)
            red = pp.tile([P, 1], FP32)

            # gv rides the head of the same queue as the x stream: it lands
            # first (~640B/partition-group) instead of being starved by
            # packet round-robin against the x stream on a second ring.
            nc.sync.dma_start(out=gv_sb[:], in_=gv[:])

            # Queue ALL x-stream DMAs up front: destination regions are
            # disjoint and never recycled, so there are no WAR stalls and the
            # SDMA engines stream HBM at line rate. Per-partition source is
            # contiguous (row r = p*NT + t layout).
            xf = x[:].rearrange("(p t) c -> p (t c)", p=P)
            ws = 0
            for T in WS:
                a, b = ws * C, (ws + T) * C
                nc.sync.dma_start(out=x_all[:, a:b], in_=xf[:, a:b])
                ws += T

            # Denominators: exp each row tile in place; the softmax row-sum
            # comes for free via the activation accumulator.
            ws = 0
            for wv, T in enumerate(WS):
                for t in range(ws, ws + T):
                    nc.scalar.activation(
                        out=x_all[:, t * C:(t + 1) * C],
                        in_=x_all[:, t * C:(t + 1) * C],
                        func=mybir.ActivationFunctionType.Exp,
                        accum_out=denom[:, t:t + 1],
                    )
                ws += T
                if wv == 0:
                    # Numerators: exp the gathered logits (one tiny ACT op,
                    # scheduled after wave 0 so gv has certainly landed),
                    # then sum each row's K entries on the vector engine.
                    nc.scalar.activation(
                        out=ge[:], in_=gv_sb[:],
                        func=mybir.ActivationFunctionType.Exp,
                    )
                    nc.vector.tensor_reduce(
                        out=numer[:],
                        in_=ge[:].rearrange("p (t k) -> p t k", k=K),
                        axis=mybir.AxisListType.X,
                        op=mybir.AluOpType.add,
                    )

            nc.vector.reciprocal(out=rec[:], in_=denom[:])
            nc.vector.tensor_tensor(
                out=loss[:], in0=numer[:], in1=rec[:], op=mybir.AluOpType.mult,
            )
            nc.vector.tensor_reduce(
                out=lsum[:], in_=loss[:],
                axis=mybir.AxisListType.X, op=mybir.AluOpType.add,
            )
            nc.gpsimd.partition_all_reduce(
                out_ap=red[:], in_ap=lsum[:], channels=P,
                reduce_op=bass_isa.ReduceOp.add,
            )
            nc.sync.dma_start(out=out[:], in_=red[:1, :])

    if not nc.is_finalized():
        nc.finalize()
    return nc


_CACHE = {}


def _prep_inputs(outputs, complementary_labels):
    outputs = np.ascontiguousarray(outputs, dtype=np.float32)
    labels = np.asarray(complementary_labels).astype(np.int64)

    in_maps = []
    for c in range(N_CORES):
        x_c = np.ascontiguousarray(outputs[c * ROWS:(c + 1) * ROWS])
        lab = labels[c * ROWS:(c + 1) * ROWS]               # [ROWS, K], row = p*NT + t
        valid = lab >= 0
        dup = np.zeros_like(valid)
        for k in range(1, K):
            dup[:, k] = (lab[:, k:k + 1] == lab[:, :k]).any(axis=1)
        keep = valid & ~dup
        safe = np.clip(lab, 0, C - 1)
        vals = np.take_along_axis(x_c, safe, axis=1)        # [ROWS, K]
        vals = np.where(keep, vals, NEG).astype(np.float32)
        gv_c = np.ascontiguousarray(vals.reshape(P, NT * K))
        in_maps.append({"x": x_c, "gv": gv_c})
    return in_maps


def kernel(outputs, complementary_labels):
    if "nc" not in _CACHE:
        _CACHE["nc"] = _build_kernel()
    nc = _CACHE["nc"]
    in_maps = _prep_inputs(outputs, complementary_labels)
    res = run_bass_kernel_spmd(nc, in_maps, list(range(N_CORES)))
    total = 0.0
    for r in res.results:
        total += float(np.asarray(r["out"]).reshape(-1)[0])
    return np.array(total / B, dtype=np.float32)


# revision 15
# speedup vs baseline: 1.0272x; 1.0272x over previous
import sys

import numpy as np

if "/opt/trn_rl_repo" not in sys.path:
    sys.path.insert(0, "/opt/trn_rl_repo")

import concourse.bacc as bacc
import concourse.bass_isa as bass_isa
import concourse.mybir as mybir
import concourse.tile as tile
from concourse.bass_utils import run_bass_kernel_spmd

# Problem constants (hardcoded per harness contract)
B, C, K = 32768, 1000, 5
N_CORES = 8
ROWS = B // N_CORES          # 4096 rows per core
P = 128                      # partitions
NT = ROWS // P               # 32 row-tiles per core; row r = p*NT + t
FP32 = mybir.dt.float32
# One DMA wave per tile: tiles land every ~1.27 µs and the scalar engine
# consumes one in ~1.20 µs (exp + pipelined accumulator read), so the exp
# pipeline tracks the per-tile completion receipts with no accumulated
# deficit — only one tile's exp remains after the last byte lands. 512 KB
# per DMA still streams at line rate on the single FIFO queue.
WS = [1] * NT
assert sum(WS) == NT
NEG = -10000.0               # exp(NEG) == 0: masks invalid/duplicate labels


def _build_kernel():
    nc = bacc.Bacc()
    x = nc.declare_dram_parameter("x", [ROWS, C], FP32, isOutput=False)
    gv = nc.declare_dram_parameter("gv", [P, NT * K], FP32, isOutput=False)
    out = nc.declare_dram_parameter("out", [1, 1], FP32, isOutput=True)

    with tile.TileContext(nc) as tc:
        with tc.tile_pool(name="pp", bufs=1) as pp:
            x_all = pp.tile([P, NT * C], FP32)   # full per-core slice, exp'd in place
            gv_sb = pp.tile([P, NT * K], FP32)   # complementary-label logits (host-gathered)
            ge = pp.tile([P, NT * K], FP32)      # exp of the above
            denom = pp.tile([P, NT], FP32)
            numer = pp.tile([P, NT], FP32)
            rec = pp.tile([P, NT], FP32)
            loss = pp.tile([P, NT], FP32)
            lsum = pp.tile([P, 1], FP32)
            red = pp.tile([P, 1], FP32)

            # gv rides the head of the same queue as the x stream: it lands
            # first (~640B/partition-group) instead of being starved by
            # packet round-robin against the x stream on a second ring.
            nc.sync.dma_start(out=gv_sb[:], in_=gv[:])

            # Queue ALL x-stream DMAs up front: destination regions are
            # disjoint and never recycled, so there are no WAR stalls and the
            # SDMA engines stream HBM at line rate. Per-partition source is
            # contiguous (row r = p*NT + t layout).
            xf = x[:].rearrange("(p t) c -> p (t c)", p=P)
            ws = 0
            for T in WS:
                a, b = ws * C, (ws + T) * C
                nc.sync.dma_start(out=x_all[:, a:b], in_=xf[:, a:b])
                ws += T

            # Denominators: exp each row tile in place; the softmax row-sum
            # comes for free via the activation accumulator.
            ws = 0
            for wv, T in enumerate(WS):
                for t in range(ws, ws + T):
                    nc.scalar.activation(
                        out=x_all[:, t * C:(t + 1) * C],
                        in_=x_all[:, t * C:(t + 1) * C],
                        func=mybir.ActivationFunctionType.Exp,
                        accum_out=denom[:, t:t + 1],
                    )
                ws += T
                if wv == 0:
                    # Numerators: exp the gathered logits (one tiny ACT op,
                    # scheduled after wave 0 so gv has certainly landed),
                    # then sum each row's K entries on the vector engine.
                    nc.scalar.activation(
                        out=ge[:], in_=gv_sb[:],
                        func=mybir.ActivationFunctionType.Exp,
                    )
                    nc.vector.tensor_reduce(
                        out=numer[:],
                        in_=ge[:].rearrange("p (t k) -> p t k", k=K),
                        axis=mybir.AxisListType.X,
                        op=mybir.AluOpType.add,
                    )

            nc.vector.reciprocal(out=rec[:], in_=denom[:])
            nc.vector.tensor_tensor(
                out=loss[:], in0=numer[:], in1=rec[:], op=mybir.AluOpType.mult,
            )
            nc.vector.tensor_reduce(
                out=lsum[:], in_=loss[:],
                axis=mybir.AxisListType.X, op=mybir.AluOpType.add,
            )
            nc.gpsimd.partition_all_reduce(
                out_ap=red[:], in_ap=lsum[:], channels=P,
                reduce_op=bass_isa.ReduceOp.add,
            )
            nc.sync.dma_start(out=out[:], in_=red[:1, :])

    if not nc.is_finalized():
        nc.finalize()
    return nc


_CACHE = {}


def _prep_inputs(outputs, complementary_labels):
    outputs = np.ascontiguousarray(outputs, dtype=np.float32)
    labels = np.asarray(complementary_labels).astype(np.int64)

    in_maps = []
    for c in range(N_CORES):
        x_c = np.ascontiguousarray(outputs[c * ROWS:(c + 1) * ROWS])
        lab = labels[c * ROWS:(c + 1) * ROWS]               # [ROWS, K], row = p*NT + t
        valid = lab >= 0
        dup = np.zeros_like(valid)
        for k in range(1, K):
            dup[:, k] = (lab[:, k:k + 1] == lab[:, :k]).any(axis=1)
        keep = valid & ~dup
        safe = np.clip(lab, 0, C - 1)
        vals = np.take_along_axis(x_c, safe, axis=1)        # [ROWS, K]
        vals = np.where(keep, vals, NEG).astype(np.float32)
        gv_c = np.ascontiguousarray(vals.reshape(P, NT * K))
        in_maps.append({"x": x_c, "gv": gv_c})
    return in_maps


def kernel(outputs, complementary_labels):
    if "nc" not in _CACHE:
        _CACHE["nc"] = _build_kernel()
    nc = _CACHE["nc"]
    in_maps = _prep_inputs(outputs, complementary_labels)
    res = run_bass_kernel_spmd(nc, in_maps, list(range(N_CORES)))
    total = 0.0
    for r in res.results:
        total += float(np.asarray(r["out"]).reshape(-1)[0])
    return np.array(total / B, dtype=np.float32)


# revision 17
# speedup vs baseline: 1.0422x; 1.0146x over previous
import sys

import numpy as np

if "/opt/trn_rl_repo" not in sys.path:
    sys.path.insert(0, "/opt/trn_rl_repo")

import concourse.bacc as bacc
import concourse.bass_isa as bass_isa
import concourse.mybir as mybir
import concourse.tile as tile
from concourse.bass_utils import run_bass_kernel_spmd

# Problem constants (hardcoded per harness contract)
B, C, K = 32768, 1000, 5
N_CORES = 8
ROWS = B // N_CORES          # 4096 rows per core
P = 128                      # partitions
NT = ROWS // P               # 32 row-tiles per core; row r = p*NT + t
FP32 = mybir.dt.float32
# One DMA wave per tile: tiles land every ~1.27 µs and the scalar engine
# consumes one in ~1.20 µs (exp + pipelined accumulator read), so the exp
# pipeline tracks the per-tile completion receipts with no accumulated
# deficit — only one tile's exp remains after the last byte lands. 512 KB
# per DMA still streams at line rate on the single FIFO queue.
WS = [1] * NT
assert sum(WS) == NT
NEG = -10000.0               # exp(NEG) == 0: masks invalid/duplicate labels


def _build_kernel():
    nc = bacc.Bacc()
    x = nc.declare_dram_parameter("x", [ROWS, C], FP32, isOutput=False)
    gv = nc.declare_dram_parameter("gv", [P, NT * K], FP32, isOutput=False)
    out = nc.declare_dram_parameter("out", [1, 1], FP32, isOutput=True)

    with tile.TileContext(nc) as tc:
        with tc.tile_pool(name="pp", bufs=1) as pp:
            x_all = pp.tile([P, NT * C], FP32)   # full per-core slice, exp'd in place
            gv_sb = pp.tile([P, NT * K], FP32)   # complementary-label logits (host-gathered)
            ge = pp.tile([P, NT * K], FP32)      # exp of the above
            denom = pp.tile([P, NT], FP32)
            numer = pp.tile([P, NT], FP32)
            rec = pp.tile([P, NT], FP32)
            loss = pp.tile([P, NT], FP32)
            lsum_a = pp.tile([P, 1], FP32)
            lsum = pp.tile([P, 1], FP32)
            red = pp.tile([P, 1], FP32)

            # gv rides the head of the same queue as the x stream: it lands
            # first (~640B/partition-group) instead of being starved by
            # packet round-robin against the x stream on a second ring.
            nc.sync.dma_start(out=gv_sb[:], in_=gv[:])

            # Queue ALL x-stream DMAs up front: destination regions are
            # disjoint and never recycled, so there are no WAR stalls and the
            # SDMA engines stream HBM at line rate. Per-partition source is
            # contiguous (row r = p*NT + t layout).
            xf = x[:].rearrange("(p t) c -> p (t c)", p=P)
            ws = 0
            for T in WS:
                a, b = ws * C, (ws + T) * C
                nc.sync.dma_start(out=x_all[:, a:b], in_=xf[:, a:b])
                ws += T

            # Denominators: exp each row tile in place; the softmax row-sum
            # comes for free via the activation accumulator.
            ws = 0
            for wv, T in enumerate(WS):
                for t in range(ws, ws + T):
                    nc.scalar.activation(
                        out=x_all[:, t * C:(t + 1) * C],
                        in_=x_all[:, t * C:(t + 1) * C],
                        func=mybir.ActivationFunctionType.Exp,
                        accum_out=denom[:, t:t + 1],
                    )
                ws += T
                if wv == 0:
                    # Numerators: exp the gathered logits (one tiny ACT op,
                    # scheduled after wave 0 so gv has certainly landed),
                    # then sum each row's K entries on the vector engine.
                    nc.scalar.activation(
                        out=ge[:], in_=gv_sb[:],
                        func=mybir.ActivationFunctionType.Exp,
                    )
                    nc.vector.tensor_reduce(
                        out=numer[:],
                        in_=ge[:].rearrange("p (t k) -> p t k", k=K),
                        axis=mybir.AxisListType.X,
                        op=mybir.AluOpType.add,
                    )
                if wv == NT - 2:
                    # Partial epilogue for tiles 0..NT-2 while the last tile
                    # still streams: only the final tile's division and a
                    # single add remain on the post-stream critical path.
                    nc.vector.reciprocal(out=rec[:, :NT - 1], in_=denom[:, :NT - 1])
                    nc.vector.tensor_tensor(
                        out=loss[:, :NT - 1], in0=numer[:, :NT - 1],
                        in1=rec[:, :NT - 1], op=mybir.AluOpType.mult,
                    )
                    nc.vector.tensor_reduce(
                        out=lsum_a[:], in_=loss[:, :NT - 1],
                        axis=mybir.AxisListType.X, op=mybir.AluOpType.add,
                    )

            nc.vector.reciprocal(out=rec[:, NT - 1:], in_=denom[:, NT - 1:])
            nc.vector.tensor_tensor(
                out=loss[:, NT - 1:], in0=numer[:, NT - 1:],
                in1=rec[:, NT - 1:], op=mybir.AluOpType.mult,
            )
            nc.vector.tensor_tensor(
                out=lsum[:], in0=lsum_a[:], in1=loss[:, NT - 1:],
                op=mybir.AluOpType.add,
            )
            nc.gpsimd.partition_all_reduce(
                out_ap=red[:], in_ap=lsum[:], channels=P,
                reduce_op=bass_isa.ReduceOp.add,
            )
            nc.sync.dma_start(out=out[:], in_=red[:1, :])

    if not nc.is_finalized():
        nc.finalize()
    return nc


_CACHE = {}


def _prep_inputs(outputs, complementary_labels):
    outputs = np.ascontiguousarray(outputs, dtype=np.float32)
    labels = np.asarray(complementary_labels).astype(np.int64)

    in_maps = []
    for c in range(N_CORES):
        x_c = np.ascontiguousarray(outputs[c * ROWS:(c + 1) * ROWS])
        lab = labels[c * ROWS:(c + 1) * ROWS]               # [ROWS, K], row = p*NT + t
        valid = lab >= 0
        dup = np.zeros_like(valid)
        for k in range(1, K):
            dup[:, k] = (lab[:, k:k + 1] == lab[:, :k]).any(axis=1)
        keep = valid & ~dup
        safe = np.clip(lab, 0, C - 1)
        vals = np.take_along_axis(x_c, safe, axis=1)        # [ROWS, K]
        vals = np.where(keep, vals, NEG).astype(np.float32)
        gv_c = np.ascontiguousarray(vals.reshape(P, NT * K))
        in_maps.append({"x": x_c, "gv": gv_c})
    return in_maps


def kernel(outputs, complementary_labels):
    if "nc" not in _CACHE:
        _CACHE["nc"] = _build_kernel()
    nc = _CACHE["nc"]
    in_maps = _prep_inputs(outputs, complementary_labels)
    res = run_bass_kernel_spmd(nc, in_maps, list(range(N_CORES)))
    total = 0.0
    for r in res.results:
        total += float(np.asarray(r["out"]).reshape(-1)[0])
    return np.array(total / B, dtype=np.float32)
